# revision 29
# baseline (speedup 1.0000x reference)
"""Entity-resolution head on 8 TRN2 NeuronCores.

Pure data-parallel: batch dim (256) split 32/core, MLP weights replicated.
All heavy tensors are bf16 (weights stream as the matmul moving operand at
1 cycle/row vs fp32's 4).  Host-side prep does the layout work: span rows
are gathered densely, first/last/pron token features are uploaded already
transposed into the lhsT layout, and only the span means (the segment
reduce) are computed on device via a masked matmul.  Every tile has a
permanent SBUF home - no buffer recycling, so the weight stream is never
back-pressured and each matmul carries a single DMA wait.
"""

import numpy as np
import ml_dtypes

import concourse.bass as bass
import concourse.mybir as mybir
import concourse.tile as tile
from concourse.bass_utils import run_bass_kernel_spmd
from concourse.masks import make_identity
from concourse.tile import add_dep_helper

B, S, H = 256, 512, 1024
HH, LH, NOUT = 512, 512, 3
EPS = 1e-5
NCORES = 8
BC = B // NCORES          # 32 batches per core
LSPAN = 15                # max span length (reference: 1..15)
KROWS = BC * LSPAN        # 480 gathered rows per span side
KPAD = 512                # padded to 4 chunks of 128
NCH = KPAD // 128         # 4
F32 = mybir.dt.float32
BF16 = mybir.dt.bfloat16
BF = ml_dtypes.bfloat16

# We1 k-chunk consumption order: host-ready feature blocks (firstA, lastA,
# firstB, lastB) first, device-computed means (meanA, meanB) last, so the
# L1e matmuls never stall on the on-device segment reduce.
# ent_emb chunk c (of 48) holds feature dims [c*128,(c+1)*128): 0-7 firstA,
# 8-15 lastA, 16-23 meanA, 24-31 firstB, 32-39 lastB, 40-47 meanB.
PERM = (list(range(0, 16))          # firstA, lastA
        + list(range(24, 40))       # firstB, lastB
        + list(range(16, 24))       # meanA
        + list(range(40, 48)))      # meanB


def _bcast_rows(ap, p):
    """AP view of a 1-D DRAM tensor broadcast across p partitions."""
    return bass.AP(tensor=ap.tensor, offset=ap.offset, ap=[[0, p]] + list(ap.ap))


def _build_program(trivial_affine, nch=NCH):
    nc = bass.Bass()

    ga_d = nc.declare_dram_parameter("ga", [128, nch, H], BF16, isOutput=False)
    gb_d = nc.declare_dram_parameter("gb", [128, nch, H], BF16, isOutput=False)
    ma_d = nc.declare_dram_parameter("ma", [128, nch, BC], BF16, isOutput=False)
    mb_d = nc.declare_dram_parameter("mb", [128, nch, BC], BF16, isOutput=False)
    stfl_d = nc.declare_dram_parameter("stfl", [128, 32, BC], BF16, isOutput=False)
    F32R = mybir.dt.float32r
    stki_d = nc.declare_dram_parameter("stki", [128, BC], F32R, isOutput=False)
    pt_d = nc.declare_dram_parameter("pt", [128, 8, BC], BF16, isOutput=False)
    wp1_d = nc.declare_dram_parameter("wp1", [128, 8, H], BF16, isOutput=False)
    we1_d = nc.declare_dram_parameter("we1", [128, 48, H], BF16, isOutput=False)
    wp2_d = nc.declare_dram_parameter("wp2", [128, 8, HH], BF16, isOutput=False)
    we2_d = nc.declare_dram_parameter("we2", [128, 8, HH], BF16, isOutput=False)
    wl_d = nc.declare_dram_parameter("wl", [128, 8, LH], BF16, isOutput=False)
    wc_d = nc.declare_dram_parameter("wc", [128, 4, NOUT], BF16, isOutput=False)
    bias_d = {}
    for name, n in [("bp1", H), ("be1", H), ("bp2", HH), ("be2", HH),
                    ("bl", LH), ("bc", NOUT)]:
        bias_d[name] = nc.declare_dram_parameter(name, [n], F32, isOutput=False)
    if not trivial_affine:
        for name, n in [("gp", H), ("betap", H), ("ge", H), ("betae", H)]:
            bias_d[name] = nc.declare_dram_parameter(name, [n], F32, isOutput=False)
    out = nc.declare_dram_parameter("out", [BC, NOUT], F32, isOutput=True)

    with tile.TileContext(nc) as tc:
        with (
            tc.tile_pool(name="singles", bufs=1) as singles,
            tc.tile_pool(name="acts", bufs=1) as acts,
            tc.tile_pool(name="psA", bufs=1, space="PSUM") as psA,
            tc.tile_pool(name="psB", bufs=1, space="PSUM") as psB,
            tc.tile_pool(name="ppart", bufs=1, space="PSUM") as ppart,
            tc.tile_pool(name="ptr", bufs=2, space="PSUM") as ptr,
        ):
            # ---------- sync helpers (walrus: one sync-wait per inst) ----
            def _raw(inst):
                return inst.ins if hasattr(inst, "ins") else inst

            def engine_absorb(eng, *dep_insts):
                """Spend drains on `eng` so it observes each producer sem;
                later same-engine instructions' auto-waits become redundant
                and are pruned, keeping every real inst at <=1 wait."""
                deps = [d for d in dep_insts if d is not None]
                dr = None
                for d in deps:
                    dr = eng.drain(fusable=False)
                    add_dep_helper(_raw(dr), _raw(d), sync=True,
                                   reason="engine observes producer")
                return dr

            def order_after(inst, dr):
                if dr is not None and inst is not None:
                    add_dep_helper(_raw(inst), _raw(dr), sync=False,
                                   reason="consumer ordered after absorber")

            # ---------- constants ----------------------------------------
            ident32 = singles.tile([32, 32], BF16, tag="ident32")
            make_identity(nc, ident32[:])
            eps_t = singles.tile([BC, 1], F32, tag="eps")
            nc.vector.memset(eps_t[:], EPS)

            # ---------- DMA streams --------------------------------------
            # gpsimd (SWDGE): gathers + small tensors FIRST (big descriptors
            #   move fastest), then held back until gathers land, then the
            #   L2/L3 weights; a few late We1 tiles
            # sync   (HWDGE): Wp1 first, then We1 tiles
            # scalar (HWDGE): We1 tiles
            ga = singles.tile([128, nch, H], BF16, tag="ga")
            gb = singles.tile([128, nch, H], BF16, tag="gb")
            ma = singles.tile([128, nch, BC], BF16, tag="ma")
            mb = singles.tile([128, nch, BC], BF16, tag="mb")
            st = singles.tile([128, 48, BC], BF16, tag="st")
            pt = singles.tile([128, 8, BC], BF16, tag="pt")
            gather_loads = []
            gather_loads.append(nc.gpsimd.dma_start(ga[:], ga_d[:]))
            gather_loads.append(nc.gpsimd.dma_start(ma[:], ma_d[:]))
            gather_loads.append(nc.gpsimd.dma_start(gb[:], gb_d[:]))
            gather_loads.append(nc.gpsimd.dma_start(mb[:], mb_d[:]))
            stfl_load = nc.gpsimd.dma_start(st[:, 0:32, :], stfl_d[:])
            pt_load = nc.gpsimd.dma_start(pt[:], pt_d[:])

            rep = {}
            rep_loads = []
            for name in bias_d:
                n = bias_d[name].shape[0]
                t = singles.tile([BC, n], F32, tag=f"rep_{name}")
                rep_loads.append(nc.gpsimd.dma_start(t[:], _bcast_rows(bias_d[name][:], BC)))
                rep[name] = t
            # absorb every bias broadcast into the DVE clock once, up front
            engine_absorb(nc.vector, *rep_loads)

            wp1 = singles.tile([128, 8, H], BF16, tag="wp1")
            wp1_loads = [nc.sync.dma_start(wp1[:, k, :], wp1_d[:, k, :])
                         for k in range(8)]

            # We1: 20 tiles each on the two HWDGE rings, last 8 on SWDGE
            # (issued after the held-back gather barrier below)
            we1 = singles.tile([128, 48, H], BF16, tag="we1")
            we1_loads = [None] * 48
            for k in range(40):
                eng = nc.sync if k % 2 == 0 else nc.scalar
                we1_loads[k] = eng.dma_start(we1[:, k, :], we1_d[:, k, :])

            # hold the SWDGE back until the early small loads are consumed:
            # a gpsimd drain observing the last gather keeps the bulk weights
            # from stealing SDMA bandwidth from the latency-critical gathers
            engine_absorb(nc.gpsimd, gather_loads[-1])
            wp2 = singles.tile([128, 8, HH], BF16, tag="wp2")
            we2 = singles.tile([128, 8, HH], BF16, tag="we2")
            wl = singles.tile([128, 8, LH], BF16, tag="wl")
            wc = singles.tile([128, 4, NOUT], BF16, tag="wc")
            wp2_load = nc.gpsimd.dma_start(wp2[:], wp2_d[:])
            for k in range(40, 48):
                we1_loads[k] = nc.gpsimd.dma_start(we1[:, k, :], we1_d[:, k, :])
            # tail-only weights stream after the last We1 tile on purpose
            we2_load = nc.gpsimd.dma_start(we2[:], we2_d[:])
            wl_load = nc.gpsimd.dma_start(wl[:], wl_d[:])
            wc_load = nc.gpsimd.dma_start(wc[:], wc_d[:])

            stki = singles.tile([128, BC], mybir.dt.float32r, tag="stki")
            stki_load = nc.gpsimd.dma_start(stki[:], stki_d[:])

            # ---------- span means (the segment reduce) ------------------
            # psm[hb][:, b] = sum_rows G[row, hb*128:...] * M[row, b]
            # The mean psum borrows the ph0 partial bank (free until L1e).
            dr = engine_absorb(nc.tensor, *gather_loads)
            psm = ppart.tile([128, 16, BC], F32, tag="ph0", name="psm")
            for si, (g_t, m_t) in enumerate(((ga, ma), (gb, mb))):
                for hb in range(8):
                    for c in range(nch):
                        mm = nc.tensor.matmul(
                            psm[:, si * 8 + hb, :],
                            lhsT=g_t[:, c, hb * 128:(hb + 1) * 128],
                            rhs=m_t[:, c, :],
                            start=(c == 0), stop=(c == nch - 1))
                        order_after(mm, dr)
            # one copy per side into the ST means blocks (bf16 cast)
            stm_a = nc.vector.tensor_copy(st[:, 32:40, :], psm[:, 0:8, :])
            stm_b = nc.vector.tensor_copy(st[:, 40:48, :], psm[:, 8:16, :])

            # ---------- packed 16-tile matmul helper ---------------------
            # Splits the K=128 reduction into 4 row-tiles and runs 4 k-chunks
            # in 4 PE column positions concurrently (K=M=32 tiles stream in
            # parallel -> ~10x per-matmul throughput).  Output lands as 4
            # partition-groups of partial sums; stki (stacked identity)
            # folds them into the batch-major psum afterwards.
            sp = {h2: singles.tile([128, HH], mybir.dt.float32r,
                                   tag=f"sp{h2}", name=f"sp{h2}")
                  for h2 in range(2)}

            def packed(ps_parts, nk, lhsT_fn, rhs_fn, dr):
                nh = len(ps_parts)
                rounds = nk // 4
                mm = None
                for r in range(rounds):
                    for h2 in range(nh):
                        for g in range(4):
                            k = 4 * r + g
                            mm = nc.tensor.matmul(
                                ps_parts[h2][32 * g:32 * g + 32, :],
                                lhsT=lhsT_fn(k),
                                rhs=rhs_fn(k, h2),
                                start=(r == 0),
                                stop=(r == rounds - 1),
                                tile_position=(0, 32 * g),
                                skip_group_check=True)
                            order_after(mm, dr)
                return mm

            def fold(ps_out, ps_parts, h2s, dep):
                """partial [128, 512] psum -> sbuf -> stki matmul -> ps_out"""
                cps = [nc.vector.tensor_copy(sp[h2][:], ps_parts[h2][:])
                       for h2 in h2s]
                dr = engine_absorb(nc.tensor, *cps, dep)
                mms = []
                for i, h2 in enumerate(h2s):
                    mm = nc.tensor.matmul(
                        ps_out[:, i * HH:(i + 1) * HH],
                        lhsT=stki[:], rhs=sp[h2][:],
                        start=True, stop=True, skip_group_check=True)
                    order_after(mm, dr)
                    mms.append(mm)
                return mms

            # ---------- L1 pron (packed) ---------------------------------
            dr = engine_absorb(nc.tensor, pt_load, stm_a, stm_b, stki_load)
            php = [ppart.tile([128, HH], F32, tag="ph0", name="l1p_h0"),
                   ppart.tile([128, HH], F32, tag="ph1", name="l1p_h1")]
            packed(php, 8,
                   lambda k: pt[:, k, :],
                   lambda k, h2: wp1[:, k, h2 * 512:(h2 + 1) * 512],
                   dr)
            ps1p = psA.tile([BC, H], F32, tag="psA", name="ps1p")
            fold(ps1p, php, (0, 1), None)

            # ---------- L1 ent (packed, all 48 chunks) -------------------
            dr = engine_absorb(nc.tensor, stfl_load)
            phe = [ppart.tile([128, HH], F32, tag="ph0", name="l1e_h0"),
                   ppart.tile([128, HH], F32, tag="ph1", name="l1e_h1")]
            packed(phe, 48,
                   lambda k: st[:, k, :],
                   lambda k, h2: we1[:, k, h2 * 512:(h2 + 1) * 512],
                   dr)

            # ---------- LN + leaky epilogue (batch-major [32, n]) --------
            def ln_leaky(ps_t, bias_t, g_t, beta_t, n, out_bf, tag):
                """x = prelu(layernorm(ps + bias) * g + beta) -> bf16."""
                x = acts.tile([BC, n], F32, tag=f"ln_{tag}")
                add = nc.vector.tensor_add(x[:], ps_t[:], bias_t[:])
                nsub = n // 512
                stats = acts.tile([BC, nsub, 6], F32, tag=f"stt_{tag}")
                xv = x[:].rearrange("p (s f) -> p s f", f=512)
                for s2 in range(nsub):
                    nc.vector.bn_stats(out=stats[:, s2, :], in_=xv[:, s2, :])
                mv = acts.tile([BC, 2], F32, tag=f"mv_{tag}")
                nc.vector.bn_aggr(out=mv[:], in_=stats[:])
                std = acts.tile([BC, 1], F32, tag=f"sd_{tag}")
                nc.scalar.activation(
                    out=std[:], in_=mv[:, 1:2],
                    func=mybir.ActivationFunctionType.Sqrt,
                    bias=eps_t[:], scale=1.0)
                rstd = acts.tile([BC, 1], F32, tag=f"rs_{tag}")
                nc.vector.reciprocal(out=rstd[:], in_=std[:])
                y = acts.tile([BC, n], F32, tag=f"y_{tag}")
                nc.vector.tensor_scalar(
                    out=y[:], in0=x[:], scalar1=mv[:, 0:1], scalar2=rstd[:],
                    op0=mybir.AluOpType.subtract, op1=mybir.AluOpType.mult)
                if g_t is not None:
                    nc.vector.tensor_mul(y[:], y[:], g_t[:])
                    nc.vector.tensor_add(y[:], y[:], beta_t[:])
                act = nc.scalar.activation(
                    out=out_bf[:], in_=y[:],
                    func=mybir.ActivationFunctionType.Prelu,
                    bias=0.0, scale=1.0, alpha=0.01)
                return act

            # LN-p issued now so the DVE/scalar work overlaps the L1e rounds
            x1p_bf = acts.tile([BC, H], BF16, tag="x1p")
            prelu_p = ln_leaky(ps1p, rep["bp1"],
                               None if trivial_affine else rep["gp"],
                               None if trivial_affine else rep["betap"],
                               H, x1p_bf, "p")

            # ---------- fold L1e partials into batch-major psum ----------
            ps1e = psB.tile([BC, H], F32, tag="psB", name="ps1e")
            fold(ps1e, phe, (0, 1), None)

            def transpose_act(src_bf, nblk, dst, dep):
                """PE-transpose batch-major [32, nblk*128] bf16 into
                feature-major [128, nblk, 32] bf16 via psum."""
                dr_t = engine_absorb(nc.tensor, dep)
                cps = []
                for hb in range(nblk):
                    pt_ps = ptr.tile([128, BC], BF16, tag="ptr")
                    mmt = nc.tensor.transpose(
                        pt_ps[:], src_bf[:, hb * 128:(hb + 1) * 128],
                        ident32[:])
                    order_after(mmt, dr_t)
                    cps.append(nc.vector.tensor_copy(dst[:, hb, :], pt_ps[:]))
                return cps

            x1pT = singles.tile([128, 8, BC], BF16, tag="x1pT")
            x1pT_cps = transpose_act(x1p_bf, 8, x1pT, prelu_p)

            # ---------- L2 pron half (runs while LN-e happens) -----------
            dr = engine_absorb(nc.tensor, *x1pT_cps)
            ps2 = psA.tile([BC, 2 * HH], F32, tag="psA", name="ps2")
            for k in range(8):
                mm = nc.tensor.matmul(
                    ps2[:, 0:HH], lhsT=x1pT[:, k, :], rhs=wp2[:, k, :],
                    start=(k == 0), stop=(k == 7))
                order_after(mm, dr)

            # ---------- LN-e + transpose + L2 ent half -------------------
            x1e_bf = acts.tile([BC, H], BF16, tag="x1e")
            prelu_e = ln_leaky(ps1e, rep["be1"],
                               None if trivial_affine else rep["ge"],
                               None if trivial_affine else rep["betae"],
                               H, x1e_bf, "e")
            x1eT = singles.tile([128, 8, BC], BF16, tag="x1eT")
            x1eT_cps = transpose_act(x1e_bf, 8, x1eT, prelu_e)

            dr = engine_absorb(nc.tensor, *x1eT_cps)
            for k in range(8):
                mm = nc.tensor.matmul(
                    ps2[:, HH:2 * HH], lhsT=x1eT[:, k, :], rhs=we2[:, k, :],
                    start=(k == 0), stop=(k == 7))
                order_after(mm, dr)

            # ---------- concat + L3 --------------------------------------
            xc_bf = acts.tile([BC, 2 * HH], BF16, tag="xc")
            a1 = nc.vector.tensor_add(xc_bf[:, 0:HH], ps2[:, 0:HH], rep["bp2"][:])
            a2 = nc.vector.tensor_add(xc_bf[:, HH:], ps2[:, HH:], rep["be2"][:])
            xcT = singles.tile([128, 8, BC], BF16, tag="xcT")
            xcT_cps = transpose_act(xc_bf, 8, xcT, a2)

            dr = engine_absorb(nc.tensor, *xcT_cps)
            ps3 = psB.tile([BC, LH], F32, tag="psB", name="ps3")
            for k in range(8):
                mm = nc.tensor.matmul(
                    ps3[:], lhsT=xcT[:, k, :], rhs=wl[:, k, :],
                    start=(k == 0), stop=(k == 7))
                order_after(mm, dr)

            # ---------- gelu (exact, hw table) ---------------------------
            g_t = acts.tile([BC, LH], F32, tag="g")
            g_add = nc.vector.tensor_add(g_t[:], ps3[:], rep["bl"][:])
            gl_bf = acts.tile([BC, LH], BF16, tag="gl")
            gelu = nc.scalar.activation(
                out=gl_bf[:], in_=g_t[:],
                func=mybir.ActivationFunctionType.Gelu,
                bias=0.0, scale=1.0)
            gT = singles.tile([128, 4, BC], BF16, tag="gT")
            gT_cps = transpose_act(gl_bf, 4, gT, gelu)

            # ---------- logits -------------------------------------------
            dr = engine_absorb(nc.tensor, *gT_cps, wc_load)
            ps4 = psB.tile([BC, NOUT], F32, tag="psB", name="ps4")
            for k in range(4):
                mm = nc.tensor.matmul(
                    ps4[:], lhsT=gT[:, k, :], rhs=wc[:, k, :],
                    start=(k == 0), stop=(k == 3))
                order_after(mm, dr)
            res = acts.tile([BC, NOUT], F32, tag="res")
            res_add = nc.vector.tensor_add(res[:], ps4[:], rep["bc"][:])
            engine_absorb(nc.sync, res_add)
            nc.sync.dma_start(out[:], res[:])

    import os
    if not os.environ.get('SKIP_PRUNE'):
        _prune_covered_waits(nc)
    nc.finalize()
    return nc


def _prune_covered_waits(nc):
    """Walrus on this toolchain accepts only one sync-wait on most
    instructions (Drain accepts many).  Within a basic block, same-engine
    instructions execute in order, so a wait already issued by an earlier
    same-engine instruction (e.g. an absorber drain) is redundant on a
    later one and can be dropped."""
    for fn in nc.m.functions:
        for blk in fn.blocks:
            insert = []
            for pos, inst in enumerate(blk.instructions):
                si = inst.sync_info
                if (inst.opcode == "Drain" and si and si.on_wait
                        and len(si.on_wait) > 1):
                    extra = list(si.on_wait[:-1])
                    si.on_wait = [si.on_wait[-1]]
                    insert.append((pos, inst, extra))
            for pos, inst, extra in reversed(insert):
                new_insts = []
                for w in extra:
                    d = mybir.InstDrain(
                        name=nc.get_next_instruction_name(),
                        ins=[], outs=[], bass_is_fusable=False)
                    d.engine = inst.engine
                    d.sync_info = mybir.SyncInfo(on_wait=[w], on_update=[])
                    nc.register_instruction(d)
                    new_insts.append(d)
                blk.instructions[pos:pos] = new_insts

    PRUNABLE = ("DMAHW", "DMASW", "PE_", "DVE_", "Pool_", "Activation_",
                "SP_")

    def prunable(w):
        return (getattr(w, "wait_mode", None) == "sem-ge-imm"
                and w.ant_name.startswith(PRUNABLE))

    for fn in nc.m.functions:
        for blk in fn.blocks:
            observed = {}
            for inst in blk.instructions:
                si = inst.sync_info
                if not si or not si.on_wait:
                    continue
                eng = str(inst.engine)
                kept = []
                for w in si.on_wait:
                    if (prunable(w)
                            and observed.get((eng, w.ant_name), -1)
                            >= w.wait_value):
                        continue
                    kept.append(w)
                for w in si.on_wait:
                    key = (eng, w.ant_name)
                    if prunable(w):
                        if observed.get(key, -1) < w.wait_value:
                            observed[key] = w.wait_value
                if len(kept) != len(si.on_wait):
                    si.on_wait = kept

    # Any remaining multi-wait non-Drain instruction: hoist all but one wait
    # onto freshly inserted same-engine Drains (Drain accepts a sync wait;
    # engine order then covers the hoisted waits for the instruction).
    for fn in nc.m.functions:
        for blk in fn.blocks:
            insert = []
            for pos, inst in enumerate(blk.instructions):
                si = inst.sync_info
                if (inst.opcode != "Drain" and si and si.on_wait
                        and len(si.on_wait) > 1):
                    extra = list(si.on_wait[:-1])
                    si.on_wait = [si.on_wait[-1]]
                    insert.append((pos, inst, extra))
            for pos, inst, extra in reversed(insert):
                new_insts = []
                for w in extra:
                    d = mybir.InstDrain(
                        name=nc.get_next_instruction_name(),
                        ins=[], outs=[], bass_is_fusable=False)
                    d.engine = inst.engine
                    d.sync_info = mybir.SyncInfo(on_wait=[w], on_update=[])
                    nc.register_instruction(d)
                    new_insts.append(d)
                blk.instructions[pos:pos] = new_insts


_PROGRAMS = {}


def _get_program(trivial_affine=True, nch=NCH):
    key = (trivial_affine, nch)
    if key not in _PROGRAMS:
        _PROGRAMS[key] = _build_program(trivial_affine, nch)
    return _PROGRAMS[key]


def _chunk_w(w, dt=BF):
    """[K, N] -> [128, K//128, N] chunk-major."""
    K, N = w.shape
    t = np.asarray(w, np.float32).reshape(K // 128, 128, N).transpose(1, 0, 2)
    return np.ascontiguousarray(t).astype(dt)


def _stacked_identity():
    """[128, 32] with ones at (32g+b, b): folds 4 psum partition-groups."""
    s = np.zeros((128, BC), np.float32)
    for g in range(4):
        s[g * BC + np.arange(BC), np.arange(BC)] = 1.0
    return s


def _feat_T(feats, dt=BF):
    """[BC, n*128] batch-major -> [128, n, BC] feature-major chunks."""
    n = feats.shape[1] // 128
    t = feats.T.reshape(n, 128, BC).transpose(1, 0, 2)
    return np.ascontiguousarray(t).astype(dt)


def make_in_maps(**inputs):
    bert = np.asarray(inputs["bert_outputs"], np.float32)
    offsets = np.asarray(inputs["offsets"], np.int32)

    trivial_affine = (
        np.all(np.asarray(inputs["gp"]) == 1.0)
        and np.all(np.asarray(inputs["betap"]) == 0.0)
        and np.all(np.asarray(inputs["ge"]) == 1.0)
        and np.all(np.asarray(inputs["betae"]) == 0.0))

    we1 = np.asarray(inputs["We1"], np.float32)
    we1_perm = we1.reshape(48, 128, H)[PERM].reshape(48 * 128, H)
    shared = {
        "wp1": _chunk_w(inputs["Wp1"]),
        "we1": _chunk_w(we1_perm),
        "wp2": _chunk_w(inputs["Wp2"]),
        "we2": _chunk_w(inputs["We2"]),
        "wl": _chunk_w(inputs["Wl"]),
        "wc": _chunk_w(inputs["Wc"]),
        "stki": _stacked_identity(),
        "bp1": np.asarray(inputs["bp1"], np.float32),
        "be1": np.asarray(inputs["be1"], np.float32),
        "bp2": np.asarray(inputs["bp2"], np.float32),
        "be2": np.asarray(inputs["be2"], np.float32),
        "bl": np.asarray(inputs["bl"], np.float32),
        "bc": np.asarray(inputs["bc"], np.float32),
    }
    if not trivial_affine:
        shared.update({
            "gp": np.asarray(inputs["gp"], np.float32),
            "betap": np.asarray(inputs["betap"], np.float32),
            "ge": np.asarray(inputs["ge"], np.float32),
            "betae": np.asarray(inputs["betae"], np.float32),
        })

    bidx = np.arange(BC)
    # spans are packed back-to-back per core side; one program is compiled
    # for the max chunk count over all cores/sides
    lens = {}
    for key, (scol, ecol) in (("A", (0, 1)), ("B", (2, 3))):
        lens[key] = (offsets[:, ecol] - offsets[:, scol]).astype(np.int64)
    rows_max = max(
        int(lens[k][c * BC:(c + 1) * BC].sum())
        for k in ("A", "B") for c in range(NCORES))
    nch = (rows_max + 127) // 128

    in_maps = []
    for c in range(NCORES):
        ob = offsets[c * BC:(c + 1) * BC]
        bc_bert = bert[c * BC:(c + 1) * BC]          # [32, S, H]
        sA, eA = ob[:, 0], ob[:, 1]
        sB, eB = ob[:, 2], ob[:, 3]
        pr = ob[:, 4]

        def side(s, e):
            ln = (e - s).astype(np.int64)
            kpad = nch * 128
            g = np.zeros((kpad, H), np.float32)
            M = np.zeros((kpad, BC), np.float32)
            pos = 0
            for b in range(BC):
                g[pos:pos + ln[b]] = bc_bert[b, s[b]:e[b]]
                M[pos:pos + ln[b], b] = 1.0 / ln[b]
                pos += ln[b]
            ga = np.ascontiguousarray(
                g.reshape(nch, 128, H).transpose(1, 0, 2)).astype(BF)
            ma = np.ascontiguousarray(
                M.reshape(nch, 128, BC).transpose(1, 0, 2)).astype(BF)
            return ga, ma

        m = dict(shared)
        m["ga"], m["ma"] = side(sA, eA)
        m["gb"], m["mb"] = side(sB, eB)
        # first/last features in ST chunk order [fA, lA, fB, lB]
        fl = np.concatenate([bc_bert[bidx, sA], bc_bert[bidx, eA - 1],
                             bc_bert[bidx, sB], bc_bert[bidx, eB - 1]], axis=1)
        m["stfl"] = _feat_T(fl)
        m["pt"] = _feat_T(bc_bert[bidx, pr])
        m["_trivial_affine"] = trivial_affine
        m["_nch"] = nch
        in_maps.append(m)
    return in_maps


def run(in_maps, **kwargs):
    trivial_affine = in_maps[0].pop("_trivial_affine", True)
    nch = in_maps[0].pop("_nch", NCH)
    for m in in_maps[1:]:
        m.pop("_trivial_affine", None)
        m.pop("_nch", None)
    nc = _get_program(trivial_affine, nch)
    return run_bass_kernel_spmd(nc, in_maps, core_ids=list(range(NCORES)), **kwargs)


def kernel(**inputs):
    res = run(make_in_maps(**inputs))
    return np.concatenate([res.results[c]["out"] for c in range(NCORES)],
                          axis=0).astype(np.float32)


# revision 30
# speedup vs baseline: 1.0233x; 1.0233x over previous
"""Entity-resolution head on 8 TRN2 NeuronCores.

Pure data-parallel: batch dim (256) split 32/core, MLP weights replicated.
All heavy tensors are bf16 (weights stream as the matmul moving operand at
1 cycle/row vs fp32's 4).  Host-side prep does the layout work: span rows
are gathered densely, first/last/pron token features are uploaded already
transposed into the lhsT layout, and only the span means (the segment
reduce) are computed on device via a masked matmul.  Every tile has a
permanent SBUF home - no buffer recycling, so the weight stream is never
back-pressured and each matmul carries a single DMA wait.
"""

import numpy as np
import ml_dtypes

import concourse.bass as bass
import concourse.mybir as mybir
import concourse.tile as tile
from concourse.bass_utils import run_bass_kernel_spmd
from concourse.masks import make_identity
from concourse.tile import add_dep_helper

B, S, H = 256, 512, 1024
HH, LH, NOUT = 512, 512, 3
EPS = 1e-5
NCORES = 8
BC = B // NCORES          # 32 batches per core
LSPAN = 15                # max span length (reference: 1..15)
KROWS = BC * LSPAN        # 480 gathered rows per span side
KPAD = 512                # padded to 4 chunks of 128
NCH = KPAD // 128         # 4
F32 = mybir.dt.float32
BF16 = mybir.dt.bfloat16
BF = ml_dtypes.bfloat16

# We1 k-chunk consumption order: host-ready feature blocks (firstA, lastA,
# firstB, lastB) first, device-computed means (meanA, meanB) last, so the
# L1e matmuls never stall on the on-device segment reduce.
# ent_emb chunk c (of 48) holds feature dims [c*128,(c+1)*128): 0-7 firstA,
# 8-15 lastA, 16-23 meanA, 24-31 firstB, 32-39 lastB, 40-47 meanB.
PERM = (list(range(0, 16))          # firstA, lastA
        + list(range(24, 40))       # firstB, lastB
        + list(range(16, 24))       # meanA
        + list(range(40, 48)))      # meanB


def _bcast_rows(ap, p):
    """AP view of a 1-D DRAM tensor broadcast across p partitions."""
    return bass.AP(tensor=ap.tensor, offset=ap.offset, ap=[[0, p]] + list(ap.ap))


def _build_program(trivial_affine, nch=NCH):
    nc = bass.Bass()

    ga_d = nc.declare_dram_parameter("ga", [128, nch, H], BF16, isOutput=False)
    gb_d = nc.declare_dram_parameter("gb", [128, nch, H], BF16, isOutput=False)
    ma_d = nc.declare_dram_parameter("ma", [128, nch, BC], BF16, isOutput=False)
    mb_d = nc.declare_dram_parameter("mb", [128, nch, BC], BF16, isOutput=False)
    stfl_d = nc.declare_dram_parameter("stfl", [128, 32, BC], BF16, isOutput=False)
    pt_d = nc.declare_dram_parameter("pt", [128, 8, BC], BF16, isOutput=False)
    wp1_d = nc.declare_dram_parameter("wp1", [128, 8, H], BF16, isOutput=False)
    we1_d = nc.declare_dram_parameter("we1", [128, 48, H], BF16, isOutput=False)
    wp2_d = nc.declare_dram_parameter("wp2", [128, 8, HH], BF16, isOutput=False)
    we2_d = nc.declare_dram_parameter("we2", [128, 8, HH], BF16, isOutput=False)
    wl_d = nc.declare_dram_parameter("wl", [128, 8, LH], BF16, isOutput=False)
    wc_d = nc.declare_dram_parameter("wc", [128, 4, NOUT], BF16, isOutput=False)
    bias_d = {}
    for name, n in [("bp1", H), ("be1", H), ("bp2", HH), ("be2", HH),
                    ("bl", LH), ("bc", NOUT)]:
        bias_d[name] = nc.declare_dram_parameter(name, [n], F32, isOutput=False)
    if not trivial_affine:
        for name, n in [("gp", H), ("betap", H), ("ge", H), ("betae", H)]:
            bias_d[name] = nc.declare_dram_parameter(name, [n], F32, isOutput=False)
    out = nc.declare_dram_parameter("out", [BC, NOUT], F32, isOutput=True)

    with tile.TileContext(nc) as tc:
        with (
            tc.tile_pool(name="singles", bufs=1) as singles,
            tc.tile_pool(name="acts", bufs=1) as acts,
            tc.tile_pool(name="psA", bufs=1, space="PSUM") as psA,
            tc.tile_pool(name="psB", bufs=1, space="PSUM") as psB,
            tc.tile_pool(name="ppart", bufs=1, space="PSUM") as ppart,
            tc.tile_pool(name="ptr", bufs=2, space="PSUM") as ptr,
        ):
            # ---------- sync helpers (walrus: one sync-wait per inst) ----
            def _raw(inst):
                return inst.ins if hasattr(inst, "ins") else inst

            def engine_absorb(eng, *dep_insts):
                """Spend drains on `eng` so it observes each producer sem;
                later same-engine instructions' auto-waits become redundant
                and are pruned, keeping every real inst at <=1 wait."""
                deps = [d for d in dep_insts if d is not None]
                dr = None
                for d in deps:
                    dr = eng.drain(fusable=False)
                    add_dep_helper(_raw(dr), _raw(d), sync=True,
                                   reason="engine observes producer")
                return dr

            def order_after(inst, dr):
                if dr is not None and inst is not None:
                    add_dep_helper(_raw(inst), _raw(dr), sync=False,
                                   reason="consumer ordered after absorber")

            # ---------- constants ----------------------------------------
            ident32 = singles.tile([32, 32], BF16, tag="ident32")
            make_identity(nc, ident32[:])
            eps_t = singles.tile([BC, 1], F32, tag="eps")
            nc.vector.memset(eps_t[:], EPS)

            # ---------- DMA streams --------------------------------------
            # gpsimd (SWDGE): gathers + small tensors FIRST (big descriptors
            #   move fastest), then held back until gathers land, then the
            #   L2/L3 weights; a few late We1 tiles
            # sync   (HWDGE): Wp1 first, then We1 tiles
            # scalar (HWDGE): We1 tiles
            ga = singles.tile([128, nch, H], BF16, tag="ga")
            gb = singles.tile([128, nch, H], BF16, tag="gb")
            ma = singles.tile([128, nch, BC], BF16, tag="ma")
            mb = singles.tile([128, nch, BC], BF16, tag="mb")
            st = singles.tile([128, 48, BC], BF16, tag="st")
            pt = singles.tile([128, 8, BC], BF16, tag="pt")
            gather_loads = []
            gather_loads.append(nc.gpsimd.dma_start(ga[:], ga_d[:]))
            gather_loads.append(nc.gpsimd.dma_start(ma[:], ma_d[:]))
            gather_loads.append(nc.gpsimd.dma_start(gb[:], gb_d[:]))
            gather_loads.append(nc.gpsimd.dma_start(mb[:], mb_d[:]))
            stfl_load = nc.gpsimd.dma_start(st[:, 0:32, :], stfl_d[:])
            pt_load = nc.gpsimd.dma_start(pt[:], pt_d[:])

            rep = {}
            rep_loads = []
            for name in bias_d:
                n = bias_d[name].shape[0]
                t = singles.tile([BC, n], F32, tag=f"rep_{name}")
                rep_loads.append(nc.gpsimd.dma_start(t[:], _bcast_rows(bias_d[name][:], BC)))
                rep[name] = t
            # absorb every bias broadcast into the DVE clock once, up front
            engine_absorb(nc.vector, *rep_loads)

            wp1 = singles.tile([128, 8, H], BF16, tag="wp1")
            wp1_loads = [nc.sync.dma_start(wp1[:, k, :], wp1_d[:, k, :])
                         for k in range(8)]

            # We1 split across both HWDGE rings: sync gets 20 (after wp1),
            # scalar gets 28, so both rings finish together
            we1 = singles.tile([128, 48, H], BF16, tag="we1")
            we1_loads = [None] * 48
            nsync = 0
            for k in range(48):
                if nsync < 20 and k % 2 == 0:
                    we1_loads[k] = nc.sync.dma_start(we1[:, k, :], we1_d[:, k, :])
                    nsync += 1
                else:
                    we1_loads[k] = nc.scalar.dma_start(we1[:, k, :], we1_d[:, k, :])

            # SWDGE sleeps until We1 is mostly delivered, then bursts the
            # L2/L3 weights while the tail of the We1 stream finishes -- the
            # big SWDGE descriptors would otherwise starve the HWDGE rings
            engine_absorb(nc.gpsimd, we1_loads[30])
            wp2 = singles.tile([128, 8, HH], BF16, tag="wp2")
            we2 = singles.tile([128, 8, HH], BF16, tag="we2")
            wl = singles.tile([128, 8, LH], BF16, tag="wl")
            wc = singles.tile([128, 4, NOUT], BF16, tag="wc")
            wp2_load = nc.gpsimd.dma_start(wp2[:], wp2_d[:])
            we2_load = nc.gpsimd.dma_start(we2[:], we2_d[:])
            wl_load = nc.gpsimd.dma_start(wl[:], wl_d[:])
            wc_load = nc.gpsimd.dma_start(wc[:], wc_d[:])

            # ---------- span means (the segment reduce) ------------------
            # psm[hb][:, b] = sum_rows G[row, hb*128:...] * M[row, b]
            # The mean psum borrows the ph0 partial bank (free until L1e).
            dr = engine_absorb(nc.tensor, *gather_loads)
            psm = ppart.tile([128, 16, BC], F32, tag="psm", name="psm")
            for si, (g_t, m_t) in enumerate(((ga, ma), (gb, mb))):
                for hb in range(8):
                    for c in range(nch):
                        mm = nc.tensor.matmul(
                            psm[:, si * 8 + hb, :],
                            lhsT=g_t[:, c, hb * 128:(hb + 1) * 128],
                            rhs=m_t[:, c, :],
                            start=(c == 0), stop=(c == nch - 1))
                        order_after(mm, dr)
            # one copy per side into the ST means blocks (bf16 cast)
            stm_a = nc.vector.tensor_copy(st[:, 32:40, :], psm[:, 0:8, :])
            stm_b = nc.vector.tensor_copy(st[:, 40:48, :], psm[:, 8:16, :])

            # ---------- L1 pron ------------------------------------------
            dr = engine_absorb(nc.tensor, pt_load, stm_a, stm_b)
            ps1p = psA.tile([BC, H], F32, tag="psA", name="ps1p")
            for k in range(8):
                for h2 in range(2):
                    mm = nc.tensor.matmul(
                        ps1p[:, h2 * 512:(h2 + 1) * 512],
                        lhsT=pt[:, k, :],
                        rhs=wp1[:, k, h2 * 512:(h2 + 1) * 512],
                        start=(k == 0), stop=(k == 7))
                    order_after(mm, dr)

            # ---------- L1 ent (all 48 chunks) ---------------------------
            dr = engine_absorb(nc.tensor, stfl_load)
            ps1e = psB.tile([BC, H], F32, tag="psB", name="ps1e")
            for k in range(48):
                for h2 in range(2):
                    mm = nc.tensor.matmul(
                        ps1e[:, h2 * 512:(h2 + 1) * 512],
                        lhsT=st[:, k, :],
                        rhs=we1[:, k, h2 * 512:(h2 + 1) * 512],
                        start=(k == 0), stop=(k == 47),
                        skip_group_check=True)
                    order_after(mm, dr)

            # ---------- LN + leaky epilogue (batch-major [32, n]) --------
            def ln_leaky(ps_t, bias_t, g_t, beta_t, n, out_bf, tag):
                """x = prelu(layernorm(ps + bias) * g + beta) -> bf16."""
                x = acts.tile([BC, n], F32, tag=f"ln_{tag}")
                add = nc.vector.tensor_add(x[:], ps_t[:], bias_t[:])
                nsub = n // 512
                stats = acts.tile([BC, nsub, 6], F32, tag=f"stt_{tag}")
                xv = x[:].rearrange("p (s f) -> p s f", f=512)
                for s2 in range(nsub):
                    nc.vector.bn_stats(out=stats[:, s2, :], in_=xv[:, s2, :])
                mv = acts.tile([BC, 2], F32, tag=f"mv_{tag}")
                nc.vector.bn_aggr(out=mv[:], in_=stats[:])
                std = acts.tile([BC, 1], F32, tag=f"sd_{tag}")
                nc.scalar.activation(
                    out=std[:], in_=mv[:, 1:2],
                    func=mybir.ActivationFunctionType.Sqrt,
                    bias=eps_t[:], scale=1.0)
                rstd = acts.tile([BC, 1], F32, tag=f"rs_{tag}")
                nc.vector.reciprocal(out=rstd[:], in_=std[:])
                y = acts.tile([BC, n], F32, tag=f"y_{tag}")
                nc.vector.tensor_scalar(
                    out=y[:], in0=x[:], scalar1=mv[:, 0:1], scalar2=rstd[:],
                    op0=mybir.AluOpType.subtract, op1=mybir.AluOpType.mult)
                if g_t is not None:
                    nc.vector.tensor_mul(y[:], y[:], g_t[:])
                    nc.vector.tensor_add(y[:], y[:], beta_t[:])
                act = nc.scalar.activation(
                    out=out_bf[:], in_=y[:],
                    func=mybir.ActivationFunctionType.Prelu,
                    bias=0.0, scale=1.0, alpha=0.01)
                return act

            # LN-p issued now so the DVE/scalar work overlaps the L1e rounds
            x1p_bf = acts.tile([BC, H], BF16, tag="x1p")
            prelu_p = ln_leaky(ps1p, rep["bp1"],
                               None if trivial_affine else rep["gp"],
                               None if trivial_affine else rep["betap"],
                               H, x1p_bf, "p")

            def transpose_act(src_bf, nblk, dst, dep):
                """PE-transpose batch-major [32, nblk*128] bf16 into
                feature-major [128, nblk, 32] bf16 via psum."""
                dr_t = engine_absorb(nc.tensor, dep)
                cps = []
                for hb in range(nblk):
                    pt_ps = ptr.tile([128, BC], BF16, tag="ptr")
                    mmt = nc.tensor.transpose(
                        pt_ps[:], src_bf[:, hb * 128:(hb + 1) * 128],
                        ident32[:])
                    order_after(mmt, dr_t)
                    cps.append(nc.vector.tensor_copy(dst[:, hb, :], pt_ps[:]))
                return cps

            x1pT = singles.tile([128, 8, BC], BF16, tag="x1pT")
            x1pT_cps = transpose_act(x1p_bf, 8, x1pT, prelu_p)

            # ---------- L2 pron half (runs while LN-e happens) -----------
            dr = engine_absorb(nc.tensor, *x1pT_cps)
            ps2 = psA.tile([BC, 2 * HH], F32, tag="psA", name="ps2")
            for k in range(8):
                mm = nc.tensor.matmul(
                    ps2[:, 0:HH], lhsT=x1pT[:, k, :], rhs=wp2[:, k, :],
                    start=(k == 0), stop=(k == 7))
                order_after(mm, dr)

            # ---------- LN-e + transpose + L2 ent half -------------------
            x1e_bf = acts.tile([BC, H], BF16, tag="x1e")
            prelu_e = ln_leaky(ps1e, rep["be1"],
                               None if trivial_affine else rep["ge"],
                               None if trivial_affine else rep["betae"],
                               H, x1e_bf, "e")
            x1eT = singles.tile([128, 8, BC], BF16, tag="x1eT")
            x1eT_cps = transpose_act(x1e_bf, 8, x1eT, prelu_e)

            dr = engine_absorb(nc.tensor, *x1eT_cps)
            for k in range(8):
                mm = nc.tensor.matmul(
                    ps2[:, HH:2 * HH], lhsT=x1eT[:, k, :], rhs=we2[:, k, :],
                    start=(k == 0), stop=(k == 7))
                order_after(mm, dr)

            # ---------- concat + L3 --------------------------------------
            xc_bf = acts.tile([BC, 2 * HH], BF16, tag="xc")
            a1 = nc.vector.tensor_add(xc_bf[:, 0:HH], ps2[:, 0:HH], rep["bp2"][:])
            a2 = nc.vector.tensor_add(xc_bf[:, HH:], ps2[:, HH:], rep["be2"][:])
            xcT = singles.tile([128, 8, BC], BF16, tag="xcT")
            xcT_cps = transpose_act(xc_bf, 8, xcT, a2)

            dr = engine_absorb(nc.tensor, *xcT_cps)
            ps3 = psB.tile([BC, LH], F32, tag="psB", name="ps3")
            for k in range(8):
                mm = nc.tensor.matmul(
                    ps3[:], lhsT=xcT[:, k, :], rhs=wl[:, k, :],
                    start=(k == 0), stop=(k == 7))
                order_after(mm, dr)

            # ---------- gelu (exact, hw table) ---------------------------
            g_t = acts.tile([BC, LH], F32, tag="g")
            g_add = nc.vector.tensor_add(g_t[:], ps3[:], rep["bl"][:])
            gl_bf = acts.tile([BC, LH], BF16, tag="gl")
            gelu = nc.scalar.activation(
                out=gl_bf[:], in_=g_t[:],
                func=mybir.ActivationFunctionType.Gelu,
                bias=0.0, scale=1.0)
            gT = singles.tile([128, 4, BC], BF16, tag="gT")
            gT_cps = transpose_act(gl_bf, 4, gT, gelu)

            # ---------- logits -------------------------------------------
            dr = engine_absorb(nc.tensor, *gT_cps, wc_load)
            ps4 = psB.tile([BC, NOUT], F32, tag="psB", name="ps4")
            for k in range(4):
                mm = nc.tensor.matmul(
                    ps4[:], lhsT=gT[:, k, :], rhs=wc[:, k, :],
                    start=(k == 0), stop=(k == 3))
                order_after(mm, dr)
            res = acts.tile([BC, NOUT], F32, tag="res")
            res_add = nc.vector.tensor_add(res[:], ps4[:], rep["bc"][:])
            engine_absorb(nc.sync, res_add)
            nc.sync.dma_start(out[:], res[:])

    import os
    if not os.environ.get('SKIP_PRUNE'):
        _prune_covered_waits(nc)
    nc.finalize()
    return nc


def _prune_covered_waits(nc):
    """Walrus on this toolchain accepts only one sync-wait on most
    instructions (Drain accepts many).  Within a basic block, same-engine
    instructions execute in order, so a wait already issued by an earlier
    same-engine instruction (e.g. an absorber drain) is redundant on a
    later one and can be dropped."""
    for fn in nc.m.functions:
        for blk in fn.blocks:
            insert = []
            for pos, inst in enumerate(blk.instructions):
                si = inst.sync_info
                if (inst.opcode == "Drain" and si and si.on_wait
                        and len(si.on_wait) > 1):
                    extra = list(si.on_wait[:-1])
                    si.on_wait = [si.on_wait[-1]]
                    insert.append((pos, inst, extra))
            for pos, inst, extra in reversed(insert):
                new_insts = []
                for w in extra:
                    d = mybir.InstDrain(
                        name=nc.get_next_instruction_name(),
                        ins=[], outs=[], bass_is_fusable=False)
                    d.engine = inst.engine
                    d.sync_info = mybir.SyncInfo(on_wait=[w], on_update=[])
                    nc.register_instruction(d)
                    new_insts.append(d)
                blk.instructions[pos:pos] = new_insts

    PRUNABLE = ("DMAHW", "DMASW", "PE_", "DVE_", "Pool_", "Activation_",
                "SP_")

    def prunable(w):
        return (getattr(w, "wait_mode", None) == "sem-ge-imm"
                and w.ant_name.startswith(PRUNABLE))

    for fn in nc.m.functions:
        for blk in fn.blocks:
            observed = {}
            for inst in blk.instructions:
                si = inst.sync_info
                if not si or not si.on_wait:
                    continue
                eng = str(inst.engine)
                kept = []
                for w in si.on_wait:
                    if (prunable(w)
                            and observed.get((eng, w.ant_name), -1)
                            >= w.wait_value):
                        continue
                    kept.append(w)
                for w in si.on_wait:
                    key = (eng, w.ant_name)
                    if prunable(w):
                        if observed.get(key, -1) < w.wait_value:
                            observed[key] = w.wait_value
                if len(kept) != len(si.on_wait):
                    si.on_wait = kept

    # Any remaining multi-wait non-Drain instruction: hoist all but one wait
    # onto freshly inserted same-engine Drains (Drain accepts a sync wait;
    # engine order then covers the hoisted waits for the instruction).
    for fn in nc.m.functions:
        for blk in fn.blocks:
            insert = []
            for pos, inst in enumerate(blk.instructions):
                si = inst.sync_info
                if (inst.opcode != "Drain" and si and si.on_wait
                        and len(si.on_wait) > 1):
                    extra = list(si.on_wait[:-1])
                    si.on_wait = [si.on_wait[-1]]
                    insert.append((pos, inst, extra))
            for pos, inst, extra in reversed(insert):
                new_insts = []
                for w in extra:
                    d = mybir.InstDrain(
                        name=nc.get_next_instruction_name(),
                        ins=[], outs=[], bass_is_fusable=False)
                    d.engine = inst.engine
                    d.sync_info = mybir.SyncInfo(on_wait=[w], on_update=[])
                    nc.register_instruction(d)
                    new_insts.append(d)
                blk.instructions[pos:pos] = new_insts


_PROGRAMS = {}


def _get_program(trivial_affine=True, nch=NCH):
    key = (trivial_affine, nch)
    if key not in _PROGRAMS:
        _PROGRAMS[key] = _build_program(trivial_affine, nch)
    return _PROGRAMS[key]


def _chunk_w(w, dt=BF):
    """[K, N] -> [128, K//128, N] chunk-major."""
    K, N = w.shape
    t = np.asarray(w, np.float32).reshape(K // 128, 128, N).transpose(1, 0, 2)
    return np.ascontiguousarray(t).astype(dt)


def _feat_T(feats, dt=BF):
    """[BC, n*128] batch-major -> [128, n, BC] feature-major chunks."""
    n = feats.shape[1] // 128
    t = feats.T.reshape(n, 128, BC).transpose(1, 0, 2)
    return np.ascontiguousarray(t).astype(dt)


def make_in_maps(**inputs):
    bert = np.asarray(inputs["bert_outputs"], np.float32)
    offsets = np.asarray(inputs["offsets"], np.int32)

    trivial_affine = (
        np.all(np.asarray(inputs["gp"]) == 1.0)
        and np.all(np.asarray(inputs["betap"]) == 0.0)
        and np.all(np.asarray(inputs["ge"]) == 1.0)
        and np.all(np.asarray(inputs["betae"]) == 0.0))

    we1 = np.asarray(inputs["We1"], np.float32)
    we1_perm = we1.reshape(48, 128, H)[PERM].reshape(48 * 128, H)
    shared = {
        "wp1": _chunk_w(inputs["Wp1"]),
        "we1": _chunk_w(we1_perm),
        "wp2": _chunk_w(inputs["Wp2"]),
        "we2": _chunk_w(inputs["We2"]),
        "wl": _chunk_w(inputs["Wl"]),
        "wc": _chunk_w(inputs["Wc"]),
        "bp1": np.asarray(inputs["bp1"], np.float32),
        "be1": np.asarray(inputs["be1"], np.float32),
        "bp2": np.asarray(inputs["bp2"], np.float32),
        "be2": np.asarray(inputs["be2"], np.float32),
        "bl": np.asarray(inputs["bl"], np.float32),
        "bc": np.asarray(inputs["bc"], np.float32),
    }
    if not trivial_affine:
        shared.update({
            "gp": np.asarray(inputs["gp"], np.float32),
            "betap": np.asarray(inputs["betap"], np.float32),
            "ge": np.asarray(inputs["ge"], np.float32),
            "betae": np.asarray(inputs["betae"], np.float32),
        })

    bidx = np.arange(BC)
    # spans are packed back-to-back per core side; one program is compiled
    # for the max chunk count over all cores/sides
    lens = {}
    for key, (scol, ecol) in (("A", (0, 1)), ("B", (2, 3))):
        lens[key] = (offsets[:, ecol] - offsets[:, scol]).astype(np.int64)
    rows_max = max(
        int(lens[k][c * BC:(c + 1) * BC].sum())
        for k in ("A", "B") for c in range(NCORES))
    nch = (rows_max + 127) // 128

    in_maps = []
    for c in range(NCORES):
        ob = offsets[c * BC:(c + 1) * BC]
        bc_bert = bert[c * BC:(c + 1) * BC]          # [32, S, H]
        sA, eA = ob[:, 0], ob[:, 1]
        sB, eB = ob[:, 2], ob[:, 3]
        pr = ob[:, 4]

        def side(s, e):
            ln = (e - s).astype(np.int64)
            kpad = nch * 128
            g = np.zeros((kpad, H), np.float32)
            M = np.zeros((kpad, BC), np.float32)
            pos = 0
            for b in range(BC):
                g[pos:pos + ln[b]] = bc_bert[b, s[b]:e[b]]
                M[pos:pos + ln[b], b] = 1.0 / ln[b]
                pos += ln[b]
            ga = np.ascontiguousarray(
                g.reshape(nch, 128, H).transpose(1, 0, 2)).astype(BF)
            ma = np.ascontiguousarray(
                M.reshape(nch, 128, BC).transpose(1, 0, 2)).astype(BF)
            return ga, ma

        m = dict(shared)
        m["ga"], m["ma"] = side(sA, eA)
        m["gb"], m["mb"] = side(sB, eB)
        # first/last features in ST chunk order [fA, lA, fB, lB]
        fl = np.concatenate([bc_bert[bidx, sA], bc_bert[bidx, eA - 1],
                             bc_bert[bidx, sB], bc_bert[bidx, eB - 1]], axis=1)
        m["stfl"] = _feat_T(fl)
        m["pt"] = _feat_T(bc_bert[bidx, pr])
        m["_trivial_affine"] = trivial_affine
        m["_nch"] = nch
        in_maps.append(m)
    return in_maps


def run(in_maps, **kwargs):
    trivial_affine = in_maps[0].pop("_trivial_affine", True)
    nch = in_maps[0].pop("_nch", NCH)
    for m in in_maps[1:]:
        m.pop("_trivial_affine", None)
        m.pop("_nch", None)
    nc = _get_program(trivial_affine, nch)
    return run_bass_kernel_spmd(nc, in_maps, core_ids=list(range(NCORES)), **kwargs)


def kernel(**inputs):
    res = run(make_in_maps(**inputs))
    return np.concatenate([res.results[c]["out"] for c in range(NCORES)],
                          axis=0).astype(np.float32)
